# revision 1
# baseline (speedup 1.0000x reference)
"""Trainium2 kernel for nn_Attention_33 (9-tile channel-attention, Restormer-style).

Strategy: the computation decomposes into 9 tiles x 4 batch = 36 fully
independent (tile, batch) work items (the attention is per-item; no
cross-item reduction).  We shard the 36 items across the 8 NeuronCores
(5 slots per core, 4 dummy slots) and run the per-item fused block on
each core; the host reassembles the 3x3 tile grid.
"""
import numpy as np

B, C, H, W = 4, 128, 384, 384
HEADS = 8
T = 9
HH, WW = H // 3, W // 3          # 128, 128
N_CORES = 8
SLOTS = 5                        # ceil(36/8)

_jit_cache = {}


def _get_runner():
    if "run" in _jit_cache:
        return _jit_cache["run"]

    import jax
    import jax.numpy as jnp
    from jax import lax

    def _item(x, ln_w, ln_b, qkv_w, qkv_b, dw_w, dw_b, proj_w, proj_b, temp, grw):
        # x: [C, HH, WW] one (tile, batch) item
        c, h, w = x.shape
        res = x
        mu = jnp.mean(x, axis=0, keepdims=True)
        var = jnp.mean((x - mu) ** 2, axis=0, keepdims=True)
        y = (x - mu) / jnp.sqrt(var + 1e-5) * ln_w[:, None, None] + ln_b[:, None, None]
        qkv = jnp.einsum('chw,oc->ohw', y, qkv_w) + qkv_b[:, None, None]
        qkv = lax.conv_general_dilated(qkv[None], dw_w, (1, 1), ((1, 1), (1, 1)),
                                       feature_group_count=3 * c,
                                       dimension_numbers=('NCHW', 'OIHW', 'NCHW'))[0]
        qkv = qkv + dw_b[:, None, None]
        q, k, v = jnp.split(qkv, 3, axis=0)
        heads = lambda t_: t_.reshape(HEADS, c // HEADS, h * w)
        q, k, v = heads(q), heads(k), heads(v)
        q = q / jnp.maximum(jnp.linalg.norm(q, axis=-1, keepdims=True), 1e-12)
        k = k / jnp.maximum(jnp.linalg.norm(k, axis=-1, keepdims=True), 1e-12)
        attn = jnp.einsum('hcn,hdn->hcd', q, k) * temp[:, None, None]
        attn = jax.nn.softmax(attn, axis=-1)
        out = jnp.einsum('hcd,hdn->hcn', attn, v).reshape(c, h, w)
        out = jnp.einsum('chw,oc->ohw', out, proj_w) + proj_b[:, None, None]
        return grw * res + out

    def _shard(xs, ln_w, ln_b, qkv_w, qkv_b, dw_w, dw_b, proj_w, proj_b, temp, grw):
        # xs: [SLOTS, C, HH, WW]; params: [SLOTS, ...]
        return jax.vmap(_item)(xs, ln_w, ln_b, qkv_w, qkv_b, dw_w, dw_b,
                               proj_w, proj_b, temp, grw)

    run = jax.pmap(_shard, axis_name='cores')
    _jit_cache["run"] = run
    return run


def kernel(x, ln_w, ln_b, qkv_w, qkv_b, dw_w, dw_b, proj_w, proj_b,
           temperature, grw):
    run = _get_runner()

    # host-side sharding: [B,C,H,W] -> [T,B,C,HH,WW] (row-major tile order)
    tiles = x.reshape(B, C, 3, HH, 3, WW).transpose(2, 4, 0, 1, 3, 5) \
             .reshape(T, B, C, HH, WW)
    items_x = tiles.reshape(T * B, C, HH, WW)            # item j = (t=j//B, b=j%B)

    t_idx = np.arange(T * B) // B                         # tile index per item
    pad = N_CORES * SLOTS - T * B                         # 4 dummy slots
    t_idx = np.concatenate([t_idx, np.zeros(pad, np.int64)])
    items_x = np.concatenate([items_x, np.zeros((pad, C, HH, WW), items_x.dtype)])

    def sh(p):  # per-item param gather -> [N_CORES, SLOTS, ...]
        g = np.ascontiguousarray(p[t_idx])
        return g.reshape(N_CORES, SLOTS, *p.shape[1:])

    xs = items_x.reshape(N_CORES, SLOTS, C, HH, WW)
    out = run(xs, sh(ln_w), sh(ln_b), sh(qkv_w), sh(qkv_b), sh(dw_w),
              sh(dw_b), sh(proj_w), sh(proj_b), sh(temperature), sh(grw))
    out = np.asarray(out).reshape(N_CORES * SLOTS, C, HH, WW)[:T * B]

    # reassemble 3x3 grid
    out = out.reshape(3, 3, B, C, HH, WW).transpose(2, 3, 0, 4, 1, 5) \
             .reshape(B, C, H, W)
    return out.astype(x.dtype)


# revision 2
# speedup vs baseline: 175.5449x; 175.5449x over previous
"""Trainium2 kernel for nn_Attention_33 (9-tile channel-attention, Restormer-style).

Strategy: the computation decomposes into 9 tiles x 4 batch = 36 fully
independent (tile, batch) work items (the attention is per-item; no
cross-item reduction).  We shard the 36 items across the 8 NeuronCores
(5 slots per core, 4 dummy slots) and run the per-item fused block on
each core; the host reassembles the 3x3 tile grid.
"""
import numpy as np

B, C, H, W = 4, 128, 384, 384
HEADS = 8
T = 9
HH, WW = H // 3, W // 3          # 128, 128
N_CORES = 8
SLOTS = 5                        # ceil(36/8)

_jit_cache = {}


def _get_runner():
    if "run" in _jit_cache:
        return _jit_cache["run"]

    import jax
    import jax.numpy as jnp
    from jax import lax

    def _item(x, ln_w, ln_b, qkv_w, qkv_b, dw_w, dw_b, proj_w, proj_b, temp, grw):
        # x: [C, HH, WW] one (tile, batch) item
        c, h, w = x.shape
        res = x
        mu = jnp.mean(x, axis=0, keepdims=True)
        var = jnp.mean((x - mu) ** 2, axis=0, keepdims=True)
        y = (x - mu) / jnp.sqrt(var + 1e-5) * ln_w[:, None, None] + ln_b[:, None, None]
        qkv = jnp.einsum('chw,oc->ohw', y, qkv_w) + qkv_b[:, None, None]
        # depthwise 3x3, padding 1, as 9 shifted multiply-adds (XLA-friendly)
        qp = jnp.pad(qkv, ((0, 0), (1, 1), (1, 1)))
        acc = dw_b[:, None, None]
        for dr in range(3):
            for dc in range(3):
                acc = acc + dw_w[:, 0, dr, dc, None, None] * \
                    lax.dynamic_slice(qp, (0, dr, dc), (3 * c, h, w))
        qkv = acc
        q, k, v = jnp.split(qkv, 3, axis=0)
        heads = lambda t_: t_.reshape(HEADS, c // HEADS, h * w)
        q, k, v = heads(q), heads(k), heads(v)
        q = q / jnp.maximum(jnp.linalg.norm(q, axis=-1, keepdims=True), 1e-12)
        k = k / jnp.maximum(jnp.linalg.norm(k, axis=-1, keepdims=True), 1e-12)
        attn = jnp.einsum('hcn,hdn->hcd', q, k) * temp[:, None, None]
        attn = jax.nn.softmax(attn, axis=-1)
        out = jnp.einsum('hcd,hdn->hcn', attn, v).reshape(c, h, w)
        out = jnp.einsum('chw,oc->ohw', out, proj_w) + proj_b[:, None, None]
        return grw * res + out

    def _shard(xs, ln_w, ln_b, qkv_w, qkv_b, dw_w, dw_b, proj_w, proj_b, temp, grw):
        # xs: [SLOTS, C, HH, WW]; params: [SLOTS, ...]
        return jax.vmap(_item)(xs, ln_w, ln_b, qkv_w, qkv_b, dw_w, dw_b,
                               proj_w, proj_b, temp, grw)

    run = jax.pmap(_shard, axis_name='cores')
    _jit_cache["run"] = run
    return run


def kernel(x, ln_w, ln_b, qkv_w, qkv_b, dw_w, dw_b, proj_w, proj_b,
           temperature, grw):
    run = _get_runner()

    # host-side sharding: [B,C,H,W] -> [T,B,C,HH,WW] (row-major tile order)
    tiles = x.reshape(B, C, 3, HH, 3, WW).transpose(2, 4, 0, 1, 3, 5) \
             .reshape(T, B, C, HH, WW)
    items_x = tiles.reshape(T * B, C, HH, WW)            # item j = (t=j//B, b=j%B)

    t_idx = np.arange(T * B) // B                         # tile index per item
    pad = N_CORES * SLOTS - T * B                         # 4 dummy slots
    t_idx = np.concatenate([t_idx, np.zeros(pad, np.int64)])
    items_x = np.concatenate([items_x, np.zeros((pad, C, HH, WW), items_x.dtype)])

    def sh(p):  # per-item param gather -> [N_CORES, SLOTS, ...]
        g = np.ascontiguousarray(p[t_idx])
        return g.reshape(N_CORES, SLOTS, *p.shape[1:])

    xs = items_x.reshape(N_CORES, SLOTS, C, HH, WW)
    out = run(xs, sh(ln_w), sh(ln_b), sh(qkv_w), sh(qkv_b), sh(dw_w),
              sh(dw_b), sh(proj_w), sh(proj_b), sh(temperature), sh(grw))
    out = np.asarray(out).reshape(N_CORES * SLOTS, C, HH, WW)[:T * B]

    # reassemble 3x3 grid
    out = out.reshape(3, 3, B, C, HH, WW).transpose(2, 3, 0, 4, 1, 5) \
             .reshape(B, C, H, W)
    return out.astype(x.dtype)
